# revision 1
# baseline (speedup 1.0000x reference)
"""Trainium2 Bass kernel for gated short-time-warp + Conv1d (nn_GW_Conv1D).

Reference computation (per batch element b, C=64 channels, T=32768):
  g = tanh(einsum('ct,c->t', x, est_w)) * 0.5            # velocity, |g| <= 0.5
  d = flow(g)    per 256-window (scaling & squaring, 4 iters), |d| <= 0.5
  xw = interp1d(x, p + d)   per window                    # forward warp
  y = conv1d(xw, conv_w, conv_b, k=3, SAME)               # channel mixing
  d_inv = flow(-g); out = interp1d(y, p + d_inv)          # inverse warp

Because |d| < 1 always (g bounded by tanh*0.5, flow doubles from 1/32 four
times), every linear interpolation touches only nearest neighbours, so the
warps become 3-term elementwise expressions with relu-split coefficients:
  out = x*(1-dn-dp) + x[-1]*dn + x[+1]*dp,  dn=relu(-d), dp=relu(d)
with dn/dp zeroed at window edges (replicating jnp.clip at the borders).

Sharding: pure data parallelism, batch b -> core b (8 cores).

Layouts per core:
  - warp stages: per-channel tiles (128 windows, 256) so the per-(window,w)
    coefficient tiles are shared by all channels with plain tensor_tensor ops.
  - conv: stacked-halves (128 = [ch 0..63 half0; ch 0..63 half1], t) so the
    channel mix is K=128 matmuls with block-diagonal weights.
  - SBUF->SBUF DMAs convert between the two (fp16 staging to save SBUF).
"""
import sys

sys.path.insert(0, "/opt/trn_rl_repo")

import numpy as np
from contextlib import ExitStack

import concourse.bass as bass
import concourse.tile as tile
from concourse import bacc, mybir
from concourse.bass_interp import get_hw_module
from concourse import bass_utils

F32 = mybir.dt.float32
F16 = mybir.dt.float16
AF = mybir.ActivationFunctionType
ALU = mybir.AluOpType

NCORES = 8
C, T, W = 64, 32768, 256
F = T // W            # 128 windows per batch element
H = T // 2            # half length (stacked-halves layout)
FLOW_ITERS = 4
G = 4                 # channels per warp group
NG = C // G


def _flow_iteration(nc, pool, d2, first):
    """One scaling-and-squaring step on d2 (128, 512) = [d_fwd | d_inv].
    d2 <- d2 + interp1d(d2, p + d2), per 256-column window."""
    dn = pool.tile([128, 512], F32, tag="fl_dn")
    dp = pool.tile([128, 512], F32, tag="fl_dp")
    nc.scalar.activation(dn[:], d2[:], AF.Relu, scale=-1.0)
    nc.scalar.activation(dp[:], d2[:], AF.Relu)
    # window-edge masking (jnp.clip at borders)
    nc.gpsimd.memset(dn[:, 0:1], 0.0)
    nc.gpsimd.memset(dn[:, 256:257], 0.0)
    nc.gpsimd.memset(dp[:, 255:256], 0.0)
    nc.gpsimd.memset(dp[:, 511:512], 0.0)
    am = pool.tile([128, 512], F32, tag="fl_am")
    nc.vector.tensor_tensor(am[:], dn[:], dp[:], ALU.add)
    nc.vector.tensor_scalar(am[:], am[:], -1.0, 1.0, ALU.mult, ALU.add)
    itp = pool.tile([128, 512], F32, tag="fl_itp")
    tmp = pool.tile([128, 512], F32, tag="fl_tmp")
    nc.vector.tensor_tensor(itp[:], d2[:], am[:], ALU.mult)
    # left-neighbour term (dn masked at window starts -> cross-window leak is *0)
    nc.vector.tensor_tensor(tmp[:, 1:512], d2[:, 0:511], dn[:, 1:512], ALU.mult)
    nc.vector.tensor_tensor(itp[:, 1:512], itp[:, 1:512], tmp[:, 1:512], ALU.add)
    # right-neighbour term
    nc.vector.tensor_tensor(tmp[:, 0:511], d2[:, 1:512], dp[:, 0:511], ALU.mult)
    nc.vector.tensor_tensor(itp[:, 0:511], itp[:, 0:511], tmp[:, 0:511], ALU.add)
    nc.vector.tensor_tensor(d2[:], d2[:], itp[:], ALU.add)
    return dn, dp


def _build_module():
    nc = bacc.Bacc("TRN2", target_bir_lowering=False, debug=False,
                   enable_asserts=False, num_devices=NCORES)
    x = nc.dram_tensor("x", (C, T), F32, kind="ExternalInput").ap()
    ew = nc.dram_tensor("ew", (128, 2), F32, kind="ExternalInput").ap()
    cw = nc.dram_tensor("cw", (128, 384), F16, kind="ExternalInput").ap()
    cb = nc.dram_tensor("cb", (128, 1), F32, kind="ExternalInput").ap()
    y = nc.dram_tensor("y", (C, T), F32, kind="ExternalOutput").ap()

    x_hc = x.rearrange("c (h t) -> h c t", h=2)          # (2, 64, H)
    x_fc = x.rearrange("c (f w) -> f c w", w=W)          # (128, 64, 256)
    y_cf = y.rearrange("c (f w) -> c f w", w=W)          # (64, 128, 256)

    with tile.TileContext(nc) as tc, ExitStack() as ctx:
        big = ctx.enter_context(tc.tile_pool(name="big", bufs=1))
        coef = ctx.enter_context(tc.tile_pool(name="coef", bufs=1))
        sm = ctx.enter_context(tc.tile_pool(name="sm", bufs=1))

        # persistent big buffers (fp16 staging for the conv layout)
        xw_st = big.tile([128, H + 2], F16)   # col 0 / col H+1 = conv halo
        yc_st = big.tile([128, H], F16)

        # ---------------- Stage A: g = einsum(x, est_w) --------------------
        ew_sb = sm.tile([128, 2], F32, tag="ew")
        nc.sync.dma_start(ew_sb[:], ew)
        g_cmp = coef.tile([128, W], F32)      # g, windows on partitions
        with tc.tile_pool(name="eins", bufs=2) as eins, \
             tc.tile_pool(name="psA", bufs=2, space="PSUM") as psA:
            for i in range(0, H, 2048):
                xe = eins.tile([128, 2048], F32, tag="xe")
                nc.sync.dma_start(xe[:], x_hc[:, :, i:i + 2048])
                pg = psA.tile([2, 2048], F32, tag="pg")
                for j in range(4):
                    nc.tensor.matmul(pg[:, j * 512:(j + 1) * 512], ew_sb[:],
                                     xe[:, j * 512:(j + 1) * 512],
                                     start=True, stop=True)
                gch = eins.tile([2, 2048], F32, tag="gch")
                nc.scalar.copy(gch[:], pg[:])
                r = i // W
                nc.sync.dma_start(g_cmp[r:r + 8, :], gch[0:1, :])
                nc.sync.dma_start(g_cmp[64 + r:64 + r + 8, :], gch[1:2, :])

        psB = ctx.enter_context(tc.tile_pool(name="psB", bufs=4, space="PSUM"))

        # ---------------- Stage B: flow + warp coefficients ----------------
        g_th = sm.tile([128, W], F32, tag="gth")
        nc.scalar.activation(g_th[:], g_cmp[:], AF.Tanh)
        d2 = sm.tile([128, 512], F32, tag="d2")           # [d_fwd | d_inv]
        nc.vector.tensor_scalar_mul(d2[:, 0:256], g_th[:], 0.5 / 16.0)
        nc.vector.tensor_scalar_mul(d2[:, 256:512], g_th[:], -0.5 / 16.0)
        for it in range(FLOW_ITERS):
            dn, dp = _flow_iteration(nc, sm, d2, it == 0)
        # final coefficients from the integrated displacement
        dn = sm.tile([128, 512], F32, tag="cf_dn")
        dp = sm.tile([128, 512], F32, tag="cf_dp")
        nc.scalar.activation(dn[:], d2[:], AF.Relu, scale=-1.0)
        nc.scalar.activation(dp[:], d2[:], AF.Relu)
        nc.gpsimd.memset(dn[:, 0:1], 0.0)
        nc.gpsimd.memset(dn[:, 256:257], 0.0)
        nc.gpsimd.memset(dp[:, 255:256], 0.0)
        nc.gpsimd.memset(dp[:, 511:512], 0.0)
        am = sm.tile([128, 512], F32, tag="cf_am")
        nc.vector.tensor_tensor(am[:], dn[:], dp[:], ALU.add)
        nc.vector.tensor_scalar(am[:], am[:], -1.0, 1.0, ALU.mult, ALU.add)

        # replicate each coefficient G times along free dim (group tiles)
        GW = G * W
        big_coefs = {}
        for name, src, off in (("af", am, 0), ("dnf", dn, 0), ("dpf", dp, 0),
                               ("ai", am, 256), ("dni", dn, 256), ("dpi", dp, 256)):
            t = coef.tile([128, GW], F32, tag=name)
            nc.scalar.copy(t[:, 0:W], src[:, off:off + W])
            rep = W
            while rep < GW:
                nc.scalar.copy(t[:, rep:2 * rep], t[:, 0:rep])
                rep *= 2
            big_coefs[name] = t
        af, dnf, dpf = big_coefs["af"], big_coefs["dnf"], big_coefs["dpf"]
        ai, dni, dpi = big_coefs["ai"], big_coefs["dni"], big_coefs["dpi"]

        # ---------------- Stage C: forward warp ----------------------------
        wrk = ctx.enter_context(tc.tile_pool(name="wrk", bufs=3))
        for g in range(NG):
            c0 = g * G
            xg = wrk.tile([128, GW], F32, tag="win")
            nc.sync.dma_start(xg[:], x_fc[:, c0:c0 + G, :])
            xw = wrk.tile([128, GW], F32, tag="wout")
            tmp = wrk.tile([128, GW], F32, tag="wtmp")
            nc.vector.tensor_tensor(xw[:], xg[:], af[:], ALU.mult)
            nc.vector.tensor_tensor(tmp[:, 1:GW], xg[:, 0:GW - 1],
                                    dnf[:, 1:GW], ALU.mult)
            nc.vector.tensor_tensor(xw[:, 1:GW], xw[:, 1:GW],
                                    tmp[:, 1:GW], ALU.add)
            nc.vector.tensor_tensor(tmp[:, 0:GW - 1], xg[:, 1:GW],
                                    dpf[:, 0:GW - 1], ALU.mult)
            nc.vector.tensor_tensor(xw[:, 0:GW - 1], xw[:, 0:GW - 1],
                                    tmp[:, 0:GW - 1], ALU.add)
            for cl in range(G):
                c = c0 + cl
                nc.gpsimd.dma_start(xw_st[c:c + 1, 1:H + 1],
                                    xw[0:64, cl * W:(cl + 1) * W])
                nc.gpsimd.dma_start(xw_st[64 + c:65 + c, 1:H + 1],
                                    xw[64:128, cl * W:(cl + 1) * W])

        # conv halo columns: half0 left pad = 0, half1 right pad = 0,
        # cross-half continuity for the interior boundary
        nc.gpsimd.memset(xw_st[0:64, 0:1], 0.0)
        nc.gpsimd.memset(xw_st[64:128, H + 1:H + 2], 0.0)
        nc.sync.dma_start(xw_st[64:128, 0:1], xw_st[0:64, H:H + 1])
        nc.sync.dma_start(xw_st[0:64, H + 1:H + 2], xw_st[64:128, 1:2])

        # ---------------- Stage D: Conv1d(C,C,3,SAME) -----------------------
        cw_sb = sm.tile([128, 384], F16, tag="cw")
        nc.sync.dma_start(cw_sb[:], cw)
        cb_sb = sm.tile([128, 1], F32, tag="cb")
        nc.sync.dma_start(cb_sb[:], cb)
        for k in range(H // 512):
            pc = psB.tile([128, 512], F32, tag="pc")
            for j in range(3):
                nc.tensor.matmul(pc[:], cw_sb[:, j * 128:(j + 1) * 128],
                                 xw_st[:, k * 512 + j:k * 512 + j + 512],
                                 start=(j == 0), stop=(j == 2))
            nc.scalar.activation(yc_st[:, k * 512:(k + 1) * 512], pc[:],
                                 AF.Identity, bias=cb_sb[:])

        # ---------------- Stage E: inverse warp + store ---------------------
        for g in range(NG):
            c0 = g * G
            yg = wrk.tile([128, GW], F32, tag="win")
            for cl in range(G):
                c = c0 + cl
                nc.gpsimd.dma_start(yg[0:64, cl * W:(cl + 1) * W],
                                    yc_st[c:c + 1, :])
                nc.gpsimd.dma_start(yg[64:128, cl * W:(cl + 1) * W],
                                    yc_st[64 + c:65 + c, :])
            yo = wrk.tile([128, GW], F32, tag="wout")
            tmp = wrk.tile([128, GW], F32, tag="wtmp")
            nc.vector.tensor_tensor(yo[:], yg[:], ai[:], ALU.mult)
            nc.vector.tensor_tensor(tmp[:, 1:GW], yg[:, 0:GW - 1],
                                    dni[:, 1:GW], ALU.mult)
            nc.vector.tensor_tensor(yo[:, 1:GW], yo[:, 1:GW],
                                    tmp[:, 1:GW], ALU.add)
            nc.vector.tensor_tensor(tmp[:, 0:GW - 1], yg[:, 1:GW],
                                    dpi[:, 0:GW - 1], ALU.mult)
            nc.vector.tensor_tensor(yo[:, 0:GW - 1], yo[:, 0:GW - 1],
                                    tmp[:, 0:GW - 1], ALU.add)
            for cl in range(G):
                nc.sync.dma_start(y_cf[c0 + cl], yo[:, cl * W:(cl + 1) * W])

    nc.compile()
    return nc


def _host_params(est_w, conv_w, conv_b):
    ew = np.zeros((128, 2), np.float32)
    ew[:64, 0] = est_w
    ew[64:, 1] = est_w
    cw = np.zeros((128, 384), np.float16)
    for j in range(3):
        blk = conv_w[:, :, j].T.astype(np.float16)   # (in, out)
        cw[0:64, j * 128:j * 128 + 64] = blk
        cw[64:128, j * 128 + 64:j * 128 + 128] = blk
    cb = np.concatenate([conv_b, conv_b]).astype(np.float32)[:, None]
    return ew, cw, cb


_COMPILED = None


def _get_compiled():
    global _COMPILED
    if _COMPILED is None:
        nc = _build_module()
        nc.m = get_hw_module(nc.m)
        _COMPILED = nc
    return _COMPILED


def kernel(signal, est_w, conv_w, conv_b, _trace=False, _trace_kwargs=None):
    nc = _get_compiled()
    ew, cw, cb = _host_params(np.asarray(est_w, np.float32),
                              np.asarray(conv_w, np.float32),
                              np.asarray(conv_b, np.float32))
    signal = np.ascontiguousarray(np.asarray(signal, np.float32))
    in_maps = [{"x": signal[b], "ew": ew, "cw": cw, "cb": cb}
               for b in range(NCORES)]
    res = bass_utils.run_bass_kernel_spmd(
        nc, in_maps, core_ids=list(range(NCORES)), trace=_trace,
        **(_trace_kwargs or {}))
    out = np.stack([r["y"] for r in res.results], axis=0)
    if _trace:
        return out, res
    return out



# revision 21
# speedup vs baseline: 1.4120x; 1.4120x over previous
"""Trainium2 Bass kernel for gated short-time-warp + Conv1d (nn_GW_Conv1D).

Reference computation (per batch element b, C=64 channels, T=32768):
  g = tanh(einsum('ct,c->t', x, est_w)) * 0.5            # velocity, |g| <= 0.5
  d = flow(g)    per 256-window (scaling & squaring, 4 iters), |d| <= 0.5
  xw = interp1d(x, p + d)   per window                    # forward warp
  y = conv1d(xw, conv_w, conv_b, k=3, SAME)               # channel mixing
  d_inv = flow(-g); out = interp1d(y, p + d_inv)          # inverse warp

|d| < 1 always, so each warp is a 3-term elementwise stencil:
  out = x*(1-dn-dp) + x[-1]*dn + x[+1]*dp,  dn=relu(-d), dp=relu(d)
with dn zeroed at window starts and dp zeroed at window ends (the clip).
The edge zeroing is folded into d itself before broadcasting:
  d[win col 0] <- max(d, 0)   (kills dn only; dp unchanged)
  d[win col W-1] <- min(d, 0) (kills dp only; dn unchanged)

v2 architecture (single streaming pass, fp16 datapath):
  - everything in "halves" layout: partition p = (h, c) (two time halves
    stacked over the 64 channels), free dim = t within half.
  - x loaded once HBM->SBUF with fp32->fp16 cast DMA (SWDGE).
  - einsum as K=128 matmuls chasing the load chunks.
  - flow on a [128 windows, 512] tile (fwd | inv side by side).
  - per-window coefficients broadcast across the 128 partitions with
    log-doubling SBUF->SBUF DMA trees (descriptor-cheap, no PSUM).
  - 6-op warp: s1=(x[-1]-x)*dn; s2=(x[+1]-x)*dp; out = x+s1+s2, on
    DVE (fp16 2x mode) with the subs offloaded to gpsimd.
  - conv as block-diagonal K=128 fp16 matmuls, bias fused in the
    scalar-engine PSUM evacuation.
  - output stored fp16 (host casts to fp32).

Sharding: pure data parallelism, batch b -> core b (8 cores).
"""
import sys

sys.path.insert(0, "/opt/trn_rl_repo")

import numpy as np
from contextlib import ExitStack

import concourse.bass as bass
import concourse.tile as tile
from concourse import bacc, mybir
from concourse.bass_interp import get_hw_module
from concourse import bass_utils

F32 = mybir.dt.float32
F16 = mybir.dt.float16
AF = mybir.ActivationFunctionType
ALU = mybir.AluOpType

NCORES = 8
C, T, W = 64, 32768, 256
H = T // 2            # 16384 cols per half
FLOW_ITERS = 4
CH = 4096             # main-loop chunk (window-aligned)
NCH = H // CH         # 4 chunks


def _flow_iteration(nc, pool, d2):
    """One scaling-and-squaring step on d2 (128, 512) = [d_fwd | d_inv].
    d2 <- d2 + interp1d(d2, p + d2), per 256-column window."""
    dn = pool.tile([128, 512], F32, tag="fl_dn")
    dp = pool.tile([128, 512], F32, tag="fl_dp")
    nc.scalar.activation(dn[:], d2[:], AF.Relu, scale=-1.0)
    nc.scalar.activation(dp[:], d2[:], AF.Relu)
    # window-edge masking (jnp.clip at borders)
    nc.gpsimd.memset(dn[:, 0:1], 0.0)
    nc.gpsimd.memset(dn[:, 256:257], 0.0)
    nc.gpsimd.memset(dp[:, 255:256], 0.0)
    nc.gpsimd.memset(dp[:, 511:512], 0.0)
    am = pool.tile([128, 512], F32, tag="fl_am")
    nc.vector.tensor_tensor(am[:], dn[:], dp[:], ALU.add)
    nc.vector.tensor_scalar(am[:], am[:], -1.0, 1.0, ALU.mult, ALU.add)
    itp = pool.tile([128, 512], F32, tag="fl_itp")
    tmp = pool.tile([128, 512], F32, tag="fl_tmp")
    nc.vector.tensor_tensor(itp[:], d2[:], am[:], ALU.mult)
    # left-neighbour term (dn masked at window starts -> cross-window leak is *0)
    nc.vector.tensor_tensor(tmp[:, 1:512], d2[:, 0:511], dn[:, 1:512], ALU.mult)
    nc.vector.tensor_tensor(itp[:, 1:512], itp[:, 1:512], tmp[:, 1:512], ALU.add)
    # right-neighbour term
    nc.vector.tensor_tensor(tmp[:, 0:511], d2[:, 1:512], dp[:, 0:511], ALU.mult)
    nc.vector.tensor_tensor(itp[:, 0:511], itp[:, 0:511], tmp[:, 0:511], ALU.add)
    nc.vector.tensor_tensor(d2[:], d2[:], itp[:], ALU.add)


def _build_module():
    nc = bacc.Bacc("TRN2", target_bir_lowering=False, debug=False,
                   enable_asserts=False, num_devices=NCORES)
    x = nc.dram_tensor("x", (C, T), F32, kind="ExternalInput").ap()
    ew = nc.dram_tensor("ew", (128, 2), F16, kind="ExternalInput").ap()
    cw = nc.dram_tensor("cw", (128, 384), F16, kind="ExternalInput").ap()
    cb = nc.dram_tensor("cb", (128, 1), F32, kind="ExternalInput").ap()
    y = nc.dram_tensor("y", (C, T), F16, kind="ExternalOutput").ap()

    # per-half views: [64, H] slices of the (C, T) tensors (outer dim 64
    # keeps DMA descriptors striped across many SDMA engines)
    x_h = [x[:, 0:H], x[:, H:T]]
    y_h = [y[:, 0:H], y[:, H:T]]

    with tile.TileContext(nc) as tc, ExitStack() as ctx:
        big = ctx.enter_context(tc.tile_pool(name="big", bufs=1))
        sm = ctx.enter_context(tc.tile_pool(name="sm", bufs=1))

        # persistent tiles: x (halo'd), warped x (halo'd), flat coef rows
        x16 = big.tile([128, H + 2], F16)
        xw = big.tile([128, H + 2], F16)
        # flat coef rows: row = coef*8 + dir*4 + h*2 + hh, cols = H/2 span hh
        flat = big.tile([16, H // 2], F16)
        nc.gpsimd.memset(x16[:, 0:1], 0.0)
        nc.gpsimd.memset(x16[:, H + 1:H + 2], 0.0)
        nc.gpsimd.memset(xw[:, 0:1], 0.0)
        nc.gpsimd.memset(xw[:, H + 1:H + 2], 0.0)

        ew_sb = sm.tile([128, 2], F16, tag="ew")
        nc.sync.dma_start(ew_sb[:], ew)
        cw_sb = sm.tile([128, 384], F16, tag="cw")
        nc.sync.dma_start(cw_sb[:], cw)
        cb_sb = sm.tile([128, 1], F32, tag="cb")
        nc.sync.dma_start(cb_sb[:], cb)

        # ------- Stage A: cast-load x, einsum g = x . est_w (chasing) -------
        # g2q rows (quarter*2 + h), cols = t within quarter of a half
        g_w = sm.tile([128, 256], F16, tag="gw")         # windows on partitions
        stageA = ctx.enter_context(tc.tile_pool(name="stA", bufs=2))
        with tc.tile_pool(name="psA", bufs=2, space="PSUM") as psA:
            for i in range(0, H, 2048):
                for h in (0, 1):
                    nc.gpsimd.dma_start(x16[h * 64:(h + 1) * 64, 1 + i:1 + i + 2048],
                                        x_h[h][:, i:i + 2048])
                pg = psA.tile([2, 2048], F32, tag="pg")
                for j in range(4):
                    nc.tensor.matmul(pg[:, j * 512:(j + 1) * 512], ew_sb[:],
                                     x16[:, 1 + i + j * 512:1 + i + (j + 1) * 512],
                                     start=True, stop=True)
                ge = stageA.tile([2, 2048], F16, tag="ge")
                nc.scalar.copy(ge[:], pg[:])
                e = i // 2048
                for h in (0, 1):
                    nc.sync.dma_start(g_w[h * 64 + e * 8:h * 64 + (e + 1) * 8, :],
                                      ge[h:h + 1, :])

        # ------- Stage B: flow -> displacement -> small coef tiles ----------
        g_th = sm.tile([128, 256], F32, tag="gth")
        nc.scalar.activation(g_th[:], g_w[:], AF.Tanh)
        d2 = sm.tile([128, 512], F32, tag="d2")          # [d_fwd | d_inv]
        nc.vector.tensor_scalar_mul(d2[:, 0:256], g_th[:], 0.5 / 16.0)
        nc.vector.tensor_scalar_mul(d2[:, 256:512], g_th[:], -0.5 / 16.0)
        for _ in range(FLOW_ITERS):
            _flow_iteration(nc, sm, d2)

        d2h = sm.tile([128, 512], F16, tag="d2h")
        nc.vector.tensor_scalar_add(d2h[:], d2[:], 0.0)
        # fold the window-edge clip into d itself
        for c0 in (0, 256):
            nc.vector.tensor_scalar_max(d2h[:, c0:c0 + 1], d2h[:, c0:c0 + 1], 0.0)
        for c0 in (255, 511):
            nc.vector.tensor_scalar_min(d2h[:, c0:c0 + 1], d2h[:, c0:c0 + 1], 0.0)
        dn_s = sm.tile([128, 512], F16, tag="dn_s")
        dp_s = sm.tile([128, 512], F16, tag="dp_s")
        nc.vector.tensor_scalar(dn_s[:], d2h[:], -1.0, 0.0, ALU.mult, ALU.max)
        nc.vector.tensor_scalar_max(dp_s[:], d2h[:], 0.0)

        # flatten to rows: flat[coef*8 + dir*4 + h*2 + hh] over H/2-col spans
        for ci_, coef_s in ((0, dn_s), (1, dp_s)):
            for dir_ in (0, 1):
                for h in (0, 1):
                    for hh in (0, 1):
                        r = ci_ * 8 + dir_ * 4 + h * 2 + hh
                        ring = nc.sync if ci_ == 0 else nc.scalar
                        ring.dma_start(
                            flat[r:r + 1, :],
                            coef_s[h * 64 + hh * 32:h * 64 + (hh + 1) * 32,
                                   dir_ * 256:(dir_ + 1) * 256])

        # ------- main streaming loop ----------------------------------------
        cf_pool = ctx.enter_context(tc.tile_pool(name="cf", bufs=2))
        ci_pool = cf_pool
        wrk = ctx.enter_context(tc.tile_pool(name="wrk", bufs=2))
        ypool = ctx.enter_context(tc.tile_pool(name="yp", bufs=2))
        psB = ctx.enter_context(tc.tile_pool(name="psB", bufs=4, space="PSUM"))

        def build_coef(pool, ring, dir_, i0, tag):
            """Broadcast coef rows to a [128, 2*CH] tile: [dn | dp]."""
            ct = pool.tile([128, 2 * CH], F16, tag=tag)
            hh, off = i0 // (H // 2), i0 % (H // 2)
            for h in (0, 1):
                for ci_ in (0, 1):
                    r = ci_ * 8 + dir_ * 4 + h * 2 + hh
                    ring.dma_start(ct[h * 64:h * 64 + 1, ci_ * CH:(ci_ + 1) * CH],
                                   flat[r:r + 1, off:off + CH])
            p = 1
            while p < 64:
                for b in (0, 64):
                    ring.dma_start(ct[b + p:b + 2 * p, :], ct[b:b + p, :])
                p *= 2
            return ct

        def warp(src, s_off, ct, dst, d_off, n):
            """dst = src + dn*(src[-1]-src) + dp*(src[+1]-src) over n cols."""
            s1 = wrk.tile([128, CH], F16, tag="s1")
            s2 = wrk.tile([128, CH], F16, tag="s2")
            nc.vector.tensor_tensor(s1[:, 0:n], src[:, s_off - 1:s_off - 1 + n],
                                    src[:, s_off:s_off + n], ALU.subtract)
            nc.gpsimd.tensor_tensor(s2[:, 0:n], src[:, s_off + 1:s_off + 1 + n],
                                    src[:, s_off:s_off + n], ALU.subtract)
            nc.vector.tensor_tensor(s1[:, 0:n], s1[:, 0:n], ct[:, 0:n], ALU.mult)
            nc.vector.tensor_tensor(s2[:, 0:n], s2[:, 0:n],
                                    ct[:, CH:CH + n], ALU.mult)
            nc.vector.tensor_tensor(dst[:, d_off:d_off + n],
                                    src[:, s_off:s_off + n], s1[:, 0:n], ALU.add)
            nc.vector.tensor_tensor(dst[:, d_off:d_off + n],
                                    dst[:, d_off:d_off + n], s2[:, 0:n], ALU.add)

        # forward warps, ordered so the cross-half conv seam is ready early
        for k in (3, 0, 1, 2):
            i0 = k * CH
            ct = build_coef(cf_pool, nc.sync, 0, i0, "ct")
            warp(x16, 1 + i0, ct, xw, 1 + i0, CH)
            if k == 3:   # half1's left conv halo = last warped col of half0
                nc.sync.dma_start(xw[64:128, 0:1], xw[0:64, H:H + 1])
            if k == 0:   # half0's right conv halo = first warped col of half1
                nc.sync.dma_start(xw[0:64, H + 1:H + 2], xw[64:128, 1:2])

        # conv + inverse warp + store, per chunk
        for k in range(NCH):
            i0 = k * CH
            y16 = ypool.tile([128, CH + 2], F16, tag="y16")
            nc.gpsimd.memset(y16[:, 0:1], 0.0)
            nc.gpsimd.memset(y16[:, CH + 1:CH + 2], 0.0)
            for b in range(CH // 512):
                pc = psB.tile([128, 512], F32, tag="pc")
                for j in range(3):
                    nc.tensor.matmul(pc[:], cw_sb[:, j * 128:(j + 1) * 128],
                                     xw[:, i0 + b * 512 + j:i0 + b * 512 + j + 512],
                                     start=(j == 0), stop=(j == 2))
                nc.scalar.activation(y16[:, 1 + b * 512:1 + (b + 1) * 512], pc[:],
                                     AF.Identity, bias=cb_sb[:])
            ci = build_coef(ci_pool, nc.scalar, 1, i0, "ct")
            warp(y16, 1, ci, y16, 1, CH)   # in-place: out = y + s1 + s2
            for h in (0, 1):
                nc.scalar.dma_start(y_h[h][:, i0:i0 + CH],
                                    y16[h * 64:(h + 1) * 64, 1:1 + CH])

    nc.compile()
    return nc


def _host_params(est_w, conv_w, conv_b):
    ew = np.zeros((128, 2), np.float16)
    ew[:64, 0] = est_w
    ew[64:, 1] = est_w
    cw = np.zeros((128, 384), np.float16)
    for j in range(3):
        blk = conv_w[:, :, j].T.astype(np.float16)   # (in, out)
        cw[0:64, j * 128:j * 128 + 64] = blk
        cw[64:128, j * 128 + 64:j * 128 + 128] = blk
    cb = np.concatenate([conv_b, conv_b]).astype(np.float32)[:, None]
    return ew, cw, cb


_COMPILED = None


def _get_compiled():
    global _COMPILED
    if _COMPILED is None:
        nc = _build_module()
        nc.m = get_hw_module(nc.m)
        _COMPILED = nc
    return _COMPILED


def kernel(signal, est_w, conv_w, conv_b, _trace=False, _trace_kwargs=None):
    nc = _get_compiled()
    ew, cw, cb = _host_params(np.asarray(est_w, np.float32),
                              np.asarray(conv_w, np.float32),
                              np.asarray(conv_b, np.float32))
    signal = np.ascontiguousarray(np.asarray(signal, np.float32))
    in_maps = [{"x": signal[b], "ew": ew, "cw": cw, "cb": cb}
               for b in range(NCORES)]
    res = bass_utils.run_bass_kernel_spmd(
        nc, in_maps, core_ids=list(range(NCORES)), trace=_trace,
        **(_trace_kwargs or {}))
    out = np.stack([np.asarray(r["y"], np.float32) for r in res.results], axis=0)
    if _trace:
        return out, res
    return out


# revision 25
# speedup vs baseline: 1.4644x; 1.0371x over previous
"""Trainium2 Bass kernel for gated short-time-warp + Conv1d (nn_GW_Conv1D).

Reference computation (per batch element b, C=64 channels, T=32768):
  g = tanh(einsum('ct,c->t', x, est_w)) * 0.5            # velocity, |g| <= 0.5
  d = flow(g)    per 256-window (scaling & squaring, 4 iters), |d| <= 0.5
  xw = interp1d(x, p + d)   per window                    # forward warp
  y = conv1d(xw, conv_w, conv_b, k=3, SAME)               # channel mixing
  d_inv = flow(-g); out = interp1d(y, p + d_inv)          # inverse warp

|d| < 1 always, so each warp is a 3-term elementwise stencil:
  out = x*(1-dn-dp) + x[-1]*dn + x[+1]*dp,  dn=relu(-d), dp=relu(d)
with dn zeroed at window starts and dp zeroed at window ends (the clip).
The edge zeroing is folded into d itself before broadcasting:
  d[win col 0] <- max(d, 0)   (kills dn only; dp unchanged)
  d[win col W-1] <- min(d, 0) (kills dp only; dn unchanged)

v2 architecture (single streaming pass, fp16 datapath):
  - everything in "halves" layout: partition p = (h, c) (two time halves
    stacked over the 64 channels), free dim = t within half.
  - x loaded once HBM->SBUF with fp32->fp16 cast DMA (SWDGE).
  - einsum as K=128 matmuls chasing the load chunks.
  - flow on a [128 windows, 512] tile (fwd | inv side by side).
  - per-window coefficients broadcast across the 128 partitions with
    log-doubling SBUF->SBUF DMA trees (descriptor-cheap, no PSUM).
  - 6-op warp: s1=(x[-1]-x)*dn; s2=(x[+1]-x)*dp; out = x+s1+s2, on
    DVE (fp16 2x mode) with the subs offloaded to gpsimd.
  - conv as block-diagonal K=128 fp16 matmuls, bias fused in the
    scalar-engine PSUM evacuation.
  - output stored fp16 (host casts to fp32).

Sharding: pure data parallelism, batch b -> core b (8 cores).
"""
import sys

sys.path.insert(0, "/opt/trn_rl_repo")

import numpy as np
from contextlib import ExitStack

import concourse.bass as bass
import concourse.tile as tile
from concourse import bacc, mybir
from concourse.bass_interp import get_hw_module
from concourse import bass_utils

F32 = mybir.dt.float32
F16 = mybir.dt.float16
AF = mybir.ActivationFunctionType
ALU = mybir.AluOpType

NCORES = 8
C, T, W = 64, 32768, 256
H = T // 2            # 16384 cols per half
FLOW_ITERS = 4
CH = 4096             # main-loop chunk (window-aligned)
NCH = H // CH         # 4 chunks


def _flow_iteration(nc, pool, d2):
    """One scaling-and-squaring step on d2 (128, 512) fp16 = [d_fwd | d_inv].
    d2 <- d2 + interp1d(d2, p + d2), per 256-column window. All-DVE to
    avoid cross-engine semaphore hops."""
    dn = pool.tile([128, 512], F16, tag="fl_dn")
    dp = pool.tile([128, 512], F16, tag="fl_dp")
    nc.vector.tensor_scalar(dn[:], d2[:], -1.0, 0.0, ALU.mult, ALU.max)
    nc.vector.tensor_scalar_max(dp[:], d2[:], 0.0)
    # window-edge masking (jnp.clip at borders)
    nc.vector.tensor_scalar_mul(dn[:, 0:1], dn[:, 0:1], 0.0)
    nc.vector.tensor_scalar_mul(dn[:, 256:257], dn[:, 256:257], 0.0)
    nc.vector.tensor_scalar_mul(dp[:, 255:256], dp[:, 255:256], 0.0)
    nc.vector.tensor_scalar_mul(dp[:, 511:512], dp[:, 511:512], 0.0)
    am = pool.tile([128, 512], F16, tag="fl_am")
    nc.vector.tensor_tensor(am[:], dn[:], dp[:], ALU.add)
    nc.vector.tensor_scalar(am[:], am[:], -1.0, 1.0, ALU.mult, ALU.add)
    itp = pool.tile([128, 512], F16, tag="fl_itp")
    tmp = pool.tile([128, 512], F16, tag="fl_tmp")
    nc.vector.tensor_tensor(itp[:], d2[:], am[:], ALU.mult)
    # left-neighbour term (dn masked at window starts -> cross-window leak is *0)
    nc.vector.tensor_tensor(tmp[:, 1:512], d2[:, 0:511], dn[:, 1:512], ALU.mult)
    nc.vector.tensor_tensor(itp[:, 1:512], itp[:, 1:512], tmp[:, 1:512], ALU.add)
    # right-neighbour term
    nc.vector.tensor_tensor(tmp[:, 0:511], d2[:, 1:512], dp[:, 0:511], ALU.mult)
    nc.vector.tensor_tensor(itp[:, 0:511], itp[:, 0:511], tmp[:, 0:511], ALU.add)
    nc.vector.tensor_tensor(d2[:], d2[:], itp[:], ALU.add)


def _build_module():
    nc = bacc.Bacc("TRN2", target_bir_lowering=False, debug=False,
                   enable_asserts=False, num_devices=NCORES)
    x = nc.dram_tensor("x", (C, T), F32, kind="ExternalInput").ap()
    ew = nc.dram_tensor("ew", (128, 2), F16, kind="ExternalInput").ap()
    cw = nc.dram_tensor("cw", (128, 384), F16, kind="ExternalInput").ap()
    cb = nc.dram_tensor("cb", (128, 1), F32, kind="ExternalInput").ap()
    y = nc.dram_tensor("y", (C, T), F16, kind="ExternalOutput").ap()

    # per-half views: [64, H] slices of the (C, T) tensors (outer dim 64
    # keeps DMA descriptors striped across many SDMA engines)
    x_h = [x[:, 0:H], x[:, H:T]]
    y_h = [y[:, 0:H], y[:, H:T]]

    with tile.TileContext(nc) as tc, ExitStack() as ctx:
        big = ctx.enter_context(tc.tile_pool(name="big", bufs=1))
        sm = ctx.enter_context(tc.tile_pool(name="sm", bufs=1))

        # persistent tiles: x (halo'd), warped x (halo'd), flat coef rows
        x16 = big.tile([128, H + 2], F16)
        xw = big.tile([128, H + 2], F16)
        # flat coef rows: row = coef*8 + dir*4 + h*2 + hh, cols = H/2 span hh
        flat = big.tile([16, H // 2], F16)
        nc.gpsimd.memset(x16[:, 0:1], 0.0)
        nc.gpsimd.memset(x16[:, H + 1:H + 2], 0.0)
        nc.gpsimd.memset(xw[:, 0:1], 0.0)
        nc.gpsimd.memset(xw[:, H + 1:H + 2], 0.0)

        ew_sb = sm.tile([128, 2], F16, tag="ew")
        nc.sync.dma_start(ew_sb[:], ew)
        cw_sb = sm.tile([128, 384], F16, tag="cw")
        nc.sync.dma_start(cw_sb[:], cw)
        cb_sb = sm.tile([128, 1], F32, tag="cb")
        nc.sync.dma_start(cb_sb[:], cb)

        # ------- Stage A: cast-load x, einsum g = x . est_w (chasing) -------
        # g2q rows (quarter*2 + h), cols = t within quarter of a half
        g_w = sm.tile([128, 256], F16, tag="gw")         # windows on partitions
        stageA = ctx.enter_context(tc.tile_pool(name="stA", bufs=2))
        # 4 big cast-load DMAs (fewer SWDGE fixed costs, fat descriptors)
        for i in (0, H // 2):
            for h in (0, 1):
                nc.gpsimd.dma_start(x16[h * 64:(h + 1) * 64, 1 + i:1 + i + H // 2],
                                    x_h[h][:, i:i + H // 2])
        with tc.tile_pool(name="psA", bufs=2, space="PSUM") as psA:
            for i in range(0, H, 2048):
                pg = psA.tile([2, 2048], F32, tag="pg")
                for j in range(4):
                    nc.tensor.matmul(pg[:, j * 512:(j + 1) * 512], ew_sb[:],
                                     x16[:, 1 + i + j * 512:1 + i + (j + 1) * 512],
                                     start=True, stop=True)
                ge = stageA.tile([2, 2048], F16, tag="ge")
                nc.scalar.copy(ge[:], pg[:])
                e = i // 2048
                for h in (0, 1):
                    nc.sync.dma_start(g_w[h * 64 + e * 8:h * 64 + (e + 1) * 8, :],
                                      ge[h:h + 1, :])

        # ------- Stage B: flow -> displacement -> small coef tiles ----------
        g_th = sm.tile([128, 256], F32, tag="gth")
        nc.scalar.activation(g_th[:], g_w[:], AF.Tanh)
        d2 = sm.tile([128, 512], F16, tag="d2")          # [d_fwd | d_inv]
        nc.vector.tensor_scalar_mul(d2[:, 0:256], g_th[:], 0.5 / 16.0)
        nc.vector.tensor_scalar_mul(d2[:, 256:512], g_th[:], -0.5 / 16.0)
        for _ in range(FLOW_ITERS):
            _flow_iteration(nc, sm, d2)

        # fold the window-edge clip into d itself
        for c0 in (0, 256):
            nc.vector.tensor_scalar_max(d2[:, c0:c0 + 1], d2[:, c0:c0 + 1], 0.0)
        for c0 in (255, 511):
            nc.vector.tensor_scalar_min(d2[:, c0:c0 + 1], d2[:, c0:c0 + 1], 0.0)
        dn_s = sm.tile([128, 512], F16, tag="dn_s")
        dp_s = sm.tile([128, 512], F16, tag="dp_s")
        nc.vector.tensor_scalar(dn_s[:], d2[:], -1.0, 0.0, ALU.mult, ALU.max)
        nc.vector.tensor_scalar_max(dp_s[:], d2[:], 0.0)

        # flatten to rows: flat[coef*8 + dir*4 + h*2 + hh] over H/2-col spans
        for ci_, coef_s in ((0, dn_s), (1, dp_s)):
            for dir_ in (0, 1):
                for h in (0, 1):
                    for hh in (0, 1):
                        r = ci_ * 8 + dir_ * 4 + h * 2 + hh
                        ring = nc.sync if ci_ == 0 else nc.scalar
                        ring.dma_start(
                            flat[r:r + 1, :],
                            coef_s[h * 64 + hh * 32:h * 64 + (hh + 1) * 32,
                                   dir_ * 256:(dir_ + 1) * 256])

        # ------- main streaming loop ----------------------------------------
        cf_pool = ctx.enter_context(tc.tile_pool(name="cf", bufs=3))
        ci_pool = cf_pool
        wrk = ctx.enter_context(tc.tile_pool(name="wrk", bufs=2))
        ypool = ctx.enter_context(tc.tile_pool(name="yp", bufs=2))
        psB = ctx.enter_context(tc.tile_pool(name="psB", bufs=4, space="PSUM"))

        def build_coef(pool, ring, dir_, i0, tag):
            """Broadcast coef rows to a [128, 2*CH] tile: [dn | dp]."""
            ct = pool.tile([128, 2 * CH], F16, tag=tag)
            hh, off = i0 // (H // 2), i0 % (H // 2)
            for h in (0, 1):
                for ci_ in (0, 1):
                    r = ci_ * 8 + dir_ * 4 + h * 2 + hh
                    ring.dma_start(ct[h * 64:h * 64 + 1, ci_ * CH:(ci_ + 1) * CH],
                                   flat[r:r + 1, off:off + CH])
            p = 1
            while p < 64:
                for b in (0, 64):
                    ring.dma_start(ct[b + p:b + 2 * p, :], ct[b:b + p, :])
                p *= 2
            return ct

        def warp(src, s_off, ct, dst, d_off, n):
            """dst = src + dn*(src[-1]-src) + dp*(src[+1]-src) over n cols."""
            s1 = wrk.tile([128, CH], F16, tag="s1")
            s2 = wrk.tile([128, CH], F16, tag="s2")
            nc.vector.tensor_tensor(s1[:, 0:n], src[:, s_off - 1:s_off - 1 + n],
                                    src[:, s_off:s_off + n], ALU.subtract)
            nc.gpsimd.tensor_tensor(s2[:, 0:n], src[:, s_off + 1:s_off + 1 + n],
                                    src[:, s_off:s_off + n], ALU.subtract)
            nc.vector.tensor_tensor(s1[:, 0:n], s1[:, 0:n], ct[:, 0:n], ALU.mult)
            nc.vector.tensor_tensor(s2[:, 0:n], s2[:, 0:n],
                                    ct[:, CH:CH + n], ALU.mult)
            nc.vector.tensor_tensor(dst[:, d_off:d_off + n],
                                    src[:, s_off:s_off + n], s1[:, 0:n], ALU.add)
            nc.vector.tensor_tensor(dst[:, d_off:d_off + n],
                                    dst[:, d_off:d_off + n], s2[:, 0:n], ALU.add)

        # forward warps, ordered so the cross-half conv seam is ready early
        for k in (3, 0, 1, 2):
            i0 = k * CH
            ct = build_coef(cf_pool, nc.sync, 0, i0, "ct")
            warp(x16, 1 + i0, ct, xw, 1 + i0, CH)
            if k == 3:   # half1's left conv halo = last warped col of half0
                nc.sync.dma_start(xw[64:128, 0:1], xw[0:64, H:H + 1])
            if k == 0:   # half0's right conv halo = first warped col of half1
                nc.sync.dma_start(xw[0:64, H + 1:H + 2], xw[64:128, 1:2])

        # conv + inverse warp + store, per chunk
        for k in range(NCH):
            i0 = k * CH
            y16 = ypool.tile([128, CH + 2], F16, tag="y16")
            nc.gpsimd.memset(y16[:, 0:1], 0.0)
            nc.gpsimd.memset(y16[:, CH + 1:CH + 2], 0.0)
            for b in range(CH // 512):
                pc = psB.tile([128, 512], F32, tag="pc")
                for j in range(3):
                    nc.tensor.matmul(pc[:], cw_sb[:, j * 128:(j + 1) * 128],
                                     xw[:, i0 + b * 512 + j:i0 + b * 512 + j + 512],
                                     start=(j == 0), stop=(j == 2))
                nc.scalar.activation(y16[:, 1 + b * 512:1 + (b + 1) * 512], pc[:],
                                     AF.Identity, bias=cb_sb[:])
            ci = build_coef(ci_pool, nc.scalar, 1, i0, "ct")
            warp(y16, 1, ci, y16, 1, CH)   # in-place: out = y + s1 + s2
            for h in (0, 1):
                nc.scalar.dma_start(y_h[h][:, i0:i0 + CH],
                                    y16[h * 64:(h + 1) * 64, 1:1 + CH])

    nc.compile()
    return nc


def _host_params(est_w, conv_w, conv_b):
    ew = np.zeros((128, 2), np.float16)
    ew[:64, 0] = est_w
    ew[64:, 1] = est_w
    cw = np.zeros((128, 384), np.float16)
    for j in range(3):
        blk = conv_w[:, :, j].T.astype(np.float16)   # (in, out)
        cw[0:64, j * 128:j * 128 + 64] = blk
        cw[64:128, j * 128 + 64:j * 128 + 128] = blk
    cb = np.concatenate([conv_b, conv_b]).astype(np.float32)[:, None]
    return ew, cw, cb


_COMPILED = None


def _get_compiled():
    global _COMPILED
    if _COMPILED is None:
        nc = _build_module()
        nc.m = get_hw_module(nc.m)
        _COMPILED = nc
    return _COMPILED


def kernel(signal, est_w, conv_w, conv_b, _trace=False, _trace_kwargs=None):
    nc = _get_compiled()
    ew, cw, cb = _host_params(np.asarray(est_w, np.float32),
                              np.asarray(conv_w, np.float32),
                              np.asarray(conv_b, np.float32))
    signal = np.ascontiguousarray(np.asarray(signal, np.float32))
    in_maps = [{"x": signal[b], "ew": ew, "cw": cw, "cb": cb}
               for b in range(NCORES)]
    res = bass_utils.run_bass_kernel_spmd(
        nc, in_maps, core_ids=list(range(NCORES)), trace=_trace,
        **(_trace_kwargs or {}))
    out = np.stack([np.asarray(r["y"], np.float32) for r in res.results], axis=0)
    if _trace:
        return out, res
    return out
